# revision 22
# baseline (speedup 1.0000x reference)
"""Trainium2 Bass kernel for nn_Conv2DMod (StyleGAN2-style modulated 3x3 conv).

Problem: x[8,64,256,256], s[8,64], weight[64,64,3,3] (f32)
  w = weight * (s+1) per sample; demod by rsqrt(sum w^2 over (Cin,K,K));
  out[b] = conv2d(x[b], w_b, pad=1).

Sharding: data-parallel over batch. 8 samples -> 8 NeuronCores, one each.

Per-core algorithm (V5):
  - host pre-pads x to [258,258] bf16 (zero border) and pre-slices the four
    row-band loads (with halos) into a [4, 128, 34, 258] tensor so every
    x DMA is one plain HWDGE transfer with 17.5KB-contiguous descriptors.
    Conv pools are opened BEFORE weight prep so the first x loads issue
    immediately (disjoint SBUF, no WAR dependency on prep tiles).
  - weight prep: f32 modulate/demodulate chain (s arrives host-replicated),
    then cast to bf16 and PE-transpose per position in bf16 (fp32 PE
    transpose runs in slow LOW_HIGH mode, ~2us each; bf16 is ~0.4us),
    replicate to partitions 64-127 via one SBUF DMA.
  - conv as shift-matmul over 9 kernel positions, PE as 4 independent
    64x64 cells. Quadrant rows = block (row-band half), quadrant cols =
    chunk parity: cell (r, c) accumulates ALL 9 positions for its own
    2-row chunk, so no cross-column adds are needed:
      unit k: psum A[0:64]=block0 rows 4k..4k+1, A[64:128]=rows 4k+2..4k+3
              psum B likewise for block1.
    Weights are loaded by the matmuls themselves (no explicit ldweights —
    walrus emits the LDWEIGHTS pair and the PE pulls it into the
    background weight buffer).
  - evacuation per unit: single full-128-partition copy psum->SBUF bf16,
    alternating scalar(ACT) / vector(DVE) engines. No tensor adds.
  - stores: bf16 output (upcast on host), on the ACT HWDGE ring while the
    x loads use the SP ring; 4 DMAs per 4-unit flush with 1KB descriptors.
"""

import numpy as np
import ml_dtypes

import concourse.bacc as bacc
import concourse.mybir as mybir
import concourse.tile as tile
from concourse.bass import ts
from concourse.bass_utils import run_bass_kernel_spmd
from concourse.masks import make_identity

F32 = mybir.dt.float32
BF16 = mybir.dt.bfloat16

B, CIN, COUT, KK, H, W = 8, 64, 64, 3, 256, 256
EPS = 1e-8
PW = W + 2          # padded row width (258)
HB = 32             # output rows per block
NBI = H // (2 * HB)  # row-band iterations (4)
NU = HB // 4        # units per iteration (8); unit = 4 rows per block
FLUSH = 2           # units per stage flush

COPY = mybir.ActivationFunctionType.Copy


def emit_prep(nc, tc, w2, wgt, s):
    """Weight prep: w2[128, 9*64] bf16 <- demodulated lhsT per position."""
    with (
        tc.tile_pool(name="prep", bufs=1) as prepp,
        tc.tile_pool(name="prep_ps", bufs=2, space="PSUM") as prep_ps,
    ):
        ident = prepp.tile([64, 64], BF16)
        make_identity(nc, ident)

        # prep DMAs ride the ACT ring: the SP ring is busy with the
        # pre-issued x loads and HWDGE rings are FIFO per engine.
        # s first: it gates the first chain op.
        s_b = prepp.tile([64, 64], F32)        # [o, i] = s[i] (host bcast)
        nc.scalar.dma_start(out=s_b[:, :], in_=s[:, :])
        w_o = prepp.tile([64, 64, 9], F32)     # [o, i, p]
        nc.scalar.dma_start(out=w_o[:, :, :], in_=wgt[:, :])
        nc.vector.tensor_scalar_add(s_b[:, :], s_b[:, :], 1.0)

        wmod = prepp.tile([64, 64, 9], F32)
        nc.vector.tensor_mul(
            wmod[:, :, :], w_o[:, :, :],
            s_b[:, :].unsqueeze(2).to_broadcast((64, 64, 9)),
        )
        sq = prepp.tile([64, 64, 9], F32)
        nc.vector.tensor_mul(sq[:, :, :], wmod[:, :, :], wmod[:, :, :])
        ssum = prepp.tile([64, 1], F32)
        nc.vector.reduce_sum(out=ssum[:, :], in_=sq[:, :, :],
                             axis=mybir.AxisListType.XY)
        epst = prepp.tile([64, 1], F32)
        nc.vector.memset(epst[:, :], EPS)
        dtmp = prepp.tile([64, 1], F32)
        nc.scalar.activation(dtmp[:, :], ssum[:, :],
                             mybir.ActivationFunctionType.Sqrt,
                             bias=epst[:, :])
        d_col = prepp.tile([64, 1], F32)
        nc.vector.reciprocal(d_col[:, :], dtmp[:, :])
        wfin = prepp.tile([64, 64, 9], BF16)   # [o, i, p] final weights, bf16
        nc.vector.tensor_scalar_mul(wfin[:, :, :], wmod[:, :, :], d_col[:, :])

        # transpose each position [o,i] -> [i,o] in bf16, into BOTH psum
        # partition halves via quadrant col-tiling, then one base-aligned
        # [128,64] copy. The conv starts consuming early positions while
        # later ones are still in prep.
        for p in range(9):
            ps_t = prep_ps.tile([128, 64], BF16, name=f"ps_t{p}", tag="ps_t")
            nc.tensor.transpose(ps_t[0:64, :], wfin[:, :, p], ident[:, :])
            nc.tensor.transpose(ps_t[64:128, :], wfin[:, :, p], ident[:, :],
                                tile_position=(0, 64))
            nc.vector.tensor_copy(w2[:, ts(p, 64)], ps_t[:, :])


def build_nc():
    nc = bacc.Bacc("TRN2")
    x = nc.dram_tensor("x", [NBI, 128, HB + 2, PW], BF16, kind="ExternalInput")
    s = nc.dram_tensor("s", [CIN, CIN], F32, kind="ExternalInput")
    wgt = nc.dram_tensor("wgt", [COUT, CIN * 9], F32, kind="ExternalInput")
    # out viewed as [Cout, row-group of 4, 4, W], bf16 (upcast on host)
    out = nc.dram_tensor("out", [COUT, H // 4, 4, W], BF16, kind="ExternalOutput")

    with tile.TileContext(nc) as tc:
        with (
            tc.tile_pool(name="const", bufs=1) as constp,
            tc.tile_pool(name="xpool", bufs=2) as xpool,
            tc.tile_pool(name="stpool", bufs=4) as stpool,
            tc.tile_pool(name="pspool", bufs=3, space="PSUM") as pspool,
        ):
            w2 = constp.tile([128, 9 * 64], BF16)

            # issue iteration-0/1 x loads first (no deps, disjoint SBUF).
            # Iterations 2/3 reuse these buffers, so their loads must be
            # emitted after the conv units that read them (sync-FIFO order).
            xts = {}
            for i in range(2):
                xts[i] = xpool.tile([128, HB + 2, PW], BF16,
                                    name=f"xt{i}", tag="xt")
            nc.sync.dma_start(out=xts[0][:, 0:6, :], in_=x[0, :, 0:6, :])
            nc.sync.dma_start(out=xts[0][:, 6:14, :], in_=x[0, :, 6:14, :])
            nc.sync.dma_start(out=xts[0][:, 14:HB + 2, :],
                              in_=x[0, :, 14:HB + 2, :])
            nc.sync.dma_start(out=xts[1][:, :, :], in_=x[1, :, :, :])

            emit_prep(nc, tc, w2, wgt, s)

            # ---- main conv loop ----
            for i in range(NBI):
                # prefetch next iteration's rows one iteration ahead so the
                # load isn't queued behind this iteration's sync-ring stores
                if i + 1 >= 2 and i + 1 < NBI:
                    xts[i + 1] = xpool.tile([128, HB + 2, PW], BF16,
                                            name=f"xt{i + 1}", tag="xt")
                    nc.sync.dma_start(out=xts[i + 1][:, :, :],
                                      in_=x[i + 1, :, :, :])
                xt = xts[i]

                for half in range(NU // FLUSH):
                    st0 = stpool.tile([128, FLUSH * 512], BF16,
                                      name=f"st0_{i}_{half}", tag="st0")
                    st1 = stpool.tile([128, FLUSH * 512], BF16,
                                      name=f"st1_{i}_{half}", tag="st1")
                    for kk in range(FLUSH):
                        k = half * FLUSH + kk
                        A = pspool.tile([128, 512], F32,
                                        name=f"A{i}_{k}", tag="A")
                        Bp = pspool.tile([128, 512], F32,
                                         name=f"B{i}_{k}", tag="B")
                        for p in range(9):
                            dy, dx = divmod(p, 3)
                            # skip_group_check: CoreSim's zero-region check
                            # is partition-unaware; HW has_written is
                            # per-element (two chains per bank on disjoint
                            # partition halves is HW-proven).
                            st = dict(start=(p == 0), stop=(p == 8),
                                      skip_group_check=True)
                            w0 = w2[0:64, ts(p, 64)]
                            w1 = w2[64:128, ts(p, 64)]
                            r0 = 4 * k + dy
                            nc.tensor.matmul(
                                A[0:64, :], w0,
                                xt[0:64, r0:r0 + 2, dx:dx + W],
                                tile_position=(0, 0), **st)
                            nc.tensor.matmul(
                                Bp[0:64, :], w1,
                                xt[64:128, r0:r0 + 2, dx:dx + W],
                                tile_position=(64, 0), **st)
                            nc.tensor.matmul(
                                A[64:128, :], w0,
                                xt[0:64, r0 + 2:r0 + 4, dx:dx + W],
                                tile_position=(0, 64), **st)
                            nc.tensor.matmul(
                                Bp[64:128, :], w1,
                                xt[64:128, r0 + 2:r0 + 4, dx:dx + W],
                                tile_position=(64, 64), **st)
                        # evacuate: one full-width copy per psum tile,
                        # alternating engines
                        d0 = st0[:, ts(kk, 512)]
                        d1 = st1[:, ts(kk, 512)]
                        if k % 2 == 0:
                            nc.scalar.activation(d0, A[:, :], COPY)
                            nc.vector.tensor_copy(d1, Bp[:, :])
                        else:
                            nc.vector.tensor_copy(d0, A[:, :])
                            nc.scalar.activation(d1, Bp[:, :], COPY)
                    # flush: 4 DMAs split across both HWDGE rings
                    g0 = 16 * i + FLUSH * half          # block0 row-groups
                    g1 = g0 + 8                         # block1 row-groups
                    nc.scalar.dma_start(out=out[:, g0:g0 + FLUSH, 0:2, :],
                                        in_=st0[0:64, :])
                    nc.sync.dma_start(out=out[:, g0:g0 + FLUSH, 2:4, :],
                                      in_=st0[64:128, :])
                    nc.scalar.dma_start(out=out[:, g1:g1 + FLUSH, 0:2, :],
                                        in_=st1[0:64, :])
                    nc.sync.dma_start(out=out[:, g1:g1 + FLUSH, 2:4, :],
                                      in_=st1[64:128, :])
    nc.finalize()
    return nc


_NC = None


def _get_nc():
    global _NC
    if _NC is None:
        _NC = build_nc()
    return _NC


def make_in_maps(x, s, weight):
    x = np.asarray(x, dtype=np.float32)
    s = np.ascontiguousarray(np.asarray(s, dtype=np.float32))
    w = np.ascontiguousarray(np.asarray(weight, dtype=np.float32)).reshape(COUT, CIN * 9)
    in_maps = []
    for c in range(B):
        xp = np.zeros((CIN, H + 2, PW), dtype=ml_dtypes.bfloat16)
        xp[:, 1:H + 1, 1:W + 1] = x[c]
        xh = np.empty((NBI, 128, HB + 2, PW), dtype=ml_dtypes.bfloat16)
        for i in range(NBI):
            xh[i, 0:64] = xp[:, 64 * i:64 * i + HB + 2, :]
            xh[i, 64:128] = xp[:, 64 * i + HB:64 * i + 2 * HB + 2, :]
        sb = np.ascontiguousarray(np.broadcast_to(s[c][None, :], (CIN, CIN)))
        in_maps.append({"x": xh, "s": sb, "wgt": w})
    return in_maps


def run(x, s, weight, **kw):
    nc = _get_nc()
    res = run_bass_kernel_spmd(nc, make_in_maps(x, s, weight),
                               core_ids=list(range(B)), **kw)
    out = np.stack([np.asarray(r["out"]).reshape(COUT, H, W)
                    for r in res.results])
    return out.astype(np.float32), res


def kernel(x, s, weight):
    out, _ = run(x, s, weight)
    return out


if __name__ == "__main__":
    rng = np.random.default_rng(0)
    xv = rng.standard_normal((B, CIN, H, W), dtype=np.float32)
    sv = rng.standard_normal((B, CIN), dtype=np.float32)
    wv = (rng.standard_normal((COUT, CIN, KK, KK), dtype=np.float32)
          * np.float32(np.sqrt(2.0 / (CIN * KK * KK))))
    o = kernel(xv, sv, wv)
    print("ran ok", o.shape, o.dtype, float(np.abs(o).max()))
